# Initial kernel scaffold
#
"""Bass/Trainium2 kernel for nn_Channel_attention (bottom-16 channel gather).

reference semantics (per sample b):
    weight = mean(x[b], axis=(H, W))           # [C]
    idx    = argsort(weight)[:16]              # ascending pooled value
    out[b] = x[b, idx]                         # [16, H, W]

Strategy: pure data parallel, B=16 sharded 2 samples per core over 8 cores.
Per core (x shard viewed as [512, 16384] = [(sample, channel), H*W]):

  Sample 0 streams into two RESIDENT [128, 16384] SBUF tiles (16 MiB).
  None of its ops ever wait on a buffer-recycle semaphore, so the Tile
  scheduler keeps its load -> reduce -> select chain first and the DMA
  queues are never head-of-line blocked.  Its output is written mid-run
  with two SWDGE indirect SCATTERS (SBUF -> DRAM, out_offset = output
  rank for selected channels, OOB-skipped otherwise), which avoids the
  1 MiB gather re-read entirely.  The two scatters target two different
  DRAM tensors ("y" and "y2") so no write-write dependency serializes
  them; the host merges rows using the stored selection indices ("sel").
  Scatter offsets are built per max8 round with an accumulating PE
  matmul so round 1's share of the work hides under round 2's DVE ops.

  Sample 1 streams through a rotating pool ([128, 2048] tiles, bufs=7);
  by the time its buffer-recycle semaphores matter, Vector has nothing
  queued but sample-1 reduces, so the recycling never throttles DMA.
  Its selection finishes last, so its 16 channels are re-fetched with
  two full-width [128, 1024] SWDGE indirect gathers (one per max8
  round, 8 ch x 16 sub-rows each; round 1's gather+store overlaps round
  2's select) and stored with direct DMAs.  The last half's chunks
  shrink so the final reduce exits quickly after the last load lands.
"""

import sys

if "/opt/trn_rl_repo" not in sys.path:
    sys.path.insert(0, "/opt/trn_rl_repo")

import numpy as np

from concourse import bacc, mybir, tile
from concourse.bass import IndirectOffsetOnAxis
from concourse.bass_utils import run_bass_kernel_spmd
from concourse.masks import make_identity

N_CORES = 8
B, C, H, W = 16, 256, 128, 128
K = 16
BPC = B // N_CORES          # samples per core = 2
E = H * W                   # 16384 elems per channel
GR = 16                     # gather sub-rows per channel (16 x 4KiB)
ROWS = BPC * C              # 512 channel rows per core
OOB = 1024                  # scatter offset for unselected channels

f32 = mybir.dt.float32
i32 = mybir.dt.int32
u32 = mybir.dt.uint32
X = mybir.AxisListType.X
Alu = mybir.AluOpType

CHUNKS_S0 = [4096] * 4                                # resident, unthrottled
CHUNKS_S1H0 = [2048] * 8                              # pooled
CHUNKS_S1H1 = [2048] * 7 + [1024, 512, 512]           # small tail chunks

_cache = {}


class _FastExitTileContext(tile.TileContext):
    """TileContext whose epilogue skips the per-semaphore clear pass.

    The standard exit drains DMA, barriers, then zeroes every allocated
    semaphore one by one (~250 sems x ~30 ns = ~7 us on the critical
    path).  That clear only matters when another kernel follows in the
    same loaded program; this kernel is single-shot per runtime session,
    so we keep the drain + one all-engine barrier and drop the clears.
    """

    def _drain_and_barrier(self, tick_clock, wait_clock):
        from concourse.vector_clock import ScopedClock

        drain_inst = self.nc.sync.drain()
        wait_clock.add_sem_waits(
            drain_inst.ins, ScopedClock({None: tick_clock.global_clock})
        )
        self.nc.all_engine_barrier()
        popped = self.nc._tile_sem_poison_stack.pop()
        assert popped is self._sem_poison


def _build():
    nc = bacc.Bacc("TRN2", target_bir_lowering=False, debug=False,
                   num_devices=N_CORES)
    x_d = nc.dram_tensor("x", [ROWS, E], f32, kind="ExternalInput")
    y_d = nc.dram_tensor("y", [BPC * K, E], f32, kind="ExternalOutput")
    y2_d = nc.dram_tensor("y2", [BPC * K, E], f32, kind="ExternalOutput")
    sel_d = nc.dram_tensor("sel", [1, K], u32, kind="ExternalOutput")

    with _FastExitTileContext(nc) as tc:
        with (
            tc.tile_pool(name="load", bufs=7) as load_pool,
            tc.tile_pool(name="small", bufs=1) as small,
            tc.tile_pool(name="psum", bufs=1, space="PSUM") as psum,
        ):
            # ---- constants (no deps; scheduler fills gaps with these) ----
            ident = small.tile([128, 128], f32)
            make_identity(nc, ident[:])

            # [8, 128] row iota 0..127 (f32) and (p>>4) variant
            row_i = small.tile([8, 128], i32)
            nc.gpsimd.iota(out=row_i[:], pattern=[[1, 128]], base=0,
                           channel_multiplier=0)
            row_f = small.tile([8, 128], f32)
            nc.vector.tensor_copy(row_f[:], row_i[:])
            rowd16_i = small.tile([8, 128], i32)
            nc.vector.tensor_scalar(out=rowd16_i[:], in0=row_i[:], scalar1=4,
                                    scalar2=None, op0=Alu.arith_shift_right)
            rowd16_f = small.tile([8, 128], f32)
            nc.vector.tensor_copy(rowd16_f[:], rowd16_i[:])

            # [8, 1] partition iota (f32)
            col8_i = small.tile([8, 1], i32)
            nc.gpsimd.iota(out=col8_i[:], pattern=[[1, 1]], base=0,
                           channel_multiplier=1)
            col8_f = small.tile([8, 1], f32)
            nc.vector.tensor_copy(col8_f[:], col8_i[:])

            # oh8_16[j, p] = (p>>4 == j): expands 8 ranks to 128 gather rows
            oh8_16 = small.tile([8, 128], f32)
            nc.vector.tensor_scalar(out=oh8_16[:], in0=rowd16_f[:],
                                    scalar1=col8_f[:], scalar2=None,
                                    op0=Alu.is_equal)

            # [128, 1] (p & 15) as f32, for gather sub-row offsets
            pp = small.tile([128, 1], i32)
            nc.gpsimd.iota(out=pp[:], pattern=[[1, 1]], base=0,
                           channel_multiplier=1)
            nc.vector.tensor_scalar(out=pp[:], in0=pp[:], scalar1=GR - 1,
                                    scalar2=None, op0=Alu.bitwise_and)
            a15f = small.tile([128, 1], f32)
            nc.vector.tensor_copy(a15f[:], pp[:])

            # rank columns for scatter offsets: rk_f[j, r] = 8r + j - OOB
            rk_f = small.tile([8, 2], f32)
            for r in range(2):
                nc.vector.tensor_scalar(out=rk_f[:, r:r + 1], in0=col8_f[:],
                                        scalar1=float(8 * r - OOB),
                                        scalar2=None, op0=Alu.add)

            xg = x_d[:].rearrange("r (u e) -> (r u) e", u=GR)
            dma_engines = [nc.sync, nc.scalar]
            state = {"n_dma": 0}

            # resident tiles for sample 0 (one per 128-channel half)
            big0 = small.tile([128, E], f32, tag="big0")
            big1 = small.tile([128, E], f32, tag="big1")
            big = [big0, big1]

            def chunk_lists(s):
                if s == 0:
                    return [CHUNKS_S0, CHUNKS_S0]
                return [CHUNKS_S1H0, CHUNKS_S1H1]

            def emit_loads(s, into_resident):
                dsts = [[], []]
                for h in range(2):
                    base = s * C + h * 128
                    off = 0
                    for cw in chunk_lists(s)[h]:
                        if into_resident:
                            dst = big[h][:, off:off + cw]
                        else:
                            t = load_pool.tile([128, 2048], f32)
                            dst = t[:, 0:cw]
                        eng = dma_engines[state["n_dma"] % 2]
                        state["n_dma"] += 1
                        eng.dma_start(out=dst,
                                      in_=x_d[base:base + 128, off:off + cw])
                        dsts[h].append(dst)
                        off += cw
                return dsts

            def emit_reduces(s, dsts):
                ncols = max(len(cl) for cl in chunk_lists(s))
                partials = small.tile([128, 2 * ncols], f32,
                                      tag=f"partials{s}")
                sums = small.tile([128, 2], f32, tag=f"sums{s}")
                psum_w = psum.tile([1, C], f32, tag=f"psw{s}")
                w_neg = small.tile([1, C], f32, tag=f"wneg{s}")
                for h in range(2):
                    cl = chunk_lists(s)[h]
                    for j, dst in enumerate(dsts[h]):
                        nc.vector.reduce_sum(
                            out=partials[:, h * ncols + j:h * ncols + j + 1],
                            in_=dst, axis=X)
                    nc.vector.reduce_sum(
                        out=sums[:, h:h + 1],
                        in_=partials[:, h * ncols:h * ncols + len(cl)],
                        axis=X, negate=True)
                    nc.tensor.matmul(out=psum_w[:, h * 128:(h + 1) * 128],
                                     lhsT=sums[:, h:h + 1], rhs=ident[:],
                                     start=True, stop=True)
                    nc.vector.tensor_copy(w_neg[:, h * 128:(h + 1) * 128],
                                          psum_w[:, h * 128:(h + 1) * 128])
                return w_neg

            def select16(s, w_neg, per_round):
                """Two max8 rounds on -sums.  per_round(r, idx_t8) runs after
                each round with that round's transposed indices [8, 1] f32.
                Returns idx_u [1, 16] u32 (ascending pooled sum)."""
                m1 = small.tile([1, 8], f32, tag=f"m1_{s}")
                m2 = small.tile([1, 8], f32, tag=f"m2_{s}")
                w_rep = small.tile([1, C], f32, tag=f"wrep{s}")
                idx_u = small.tile([1, K], u32, tag=f"idxu{s}")

                def round_tail(r):
                    idx_f = small.tile([1, 8], f32, tag=f"idxf{s}_{r}")
                    nc.vector.tensor_copy(idx_f[:],
                                          idx_u[:, 8 * r:8 * r + 8])
                    psum_t = psum.tile([8, 1], f32, tag=f"pst{s}")
                    nc.tensor.matmul(out=psum_t[:], lhsT=idx_f[:],
                                     rhs=ident[0:1, 0:1], start=True,
                                     stop=True)
                    idx_t8 = small.tile([8, 1], f32, tag=f"idxt{s}_{r}")
                    nc.vector.tensor_copy(idx_t8[:], psum_t[:])
                    per_round(r, idx_t8)

                nc.vector.max(out=m1[:], in_=w_neg[:])
                nc.vector.max_index(out=idx_u[:, 0:8], in_max=m1[:],
                                    in_values=w_neg[:])
                nc.vector.match_replace(out=w_rep[:], in_to_replace=m1[:],
                                        in_values=w_neg[:], imm_value=-1e38)
                round_tail(0)
                nc.vector.max(out=m2[:], in_=w_rep[:])
                nc.vector.max_index(out=idx_u[:, 8:16], in_max=m2[:],
                                    in_values=w_rep[:])
                round_tail(1)
                return idx_u

            # ------------- sample 0: resident loads + reduces ---------------
            dsts0 = emit_loads(0, into_resident=True)
            w0 = emit_reduces(0, dsts0)

            # sample 1 pooled load issues queue next on sync/scalar
            dsts1 = emit_loads(1, into_resident=False)

            # ------------- sample 0: select + mid-run scatters --------------
            psum_o0 = psum.tile([128, 1], f32, tag="pso0")
            psum_o1 = psum.tile([128, 1], f32, tag="pso1")
            psum_o = [psum_o0, psum_o1]

            def s0_round(r, idx_t8):
                # accumulate per-half scatter offsets:
                # offs[p] = rank(p) - OOB contribution if channel p selected
                for h in range(2):
                    if h == 0:
                        idx_cmp = idx_t8
                    else:
                        idx_cmp = small.tile([8, 1], f32, tag=f"idxsh{r}")
                        nc.vector.tensor_scalar(out=idx_cmp[:], in0=idx_t8[:],
                                                scalar1=-128.0, scalar2=None,
                                                op0=Alu.add)
                    oh = small.tile([8, 128], f32, tag=f"oh{h}_{r}")
                    nc.vector.tensor_scalar(out=oh[:], in0=row_f[:],
                                            scalar1=idx_cmp[:], scalar2=None,
                                            op0=Alu.is_equal)
                    nc.tensor.matmul(out=psum_o[h][:], lhsT=oh[:],
                                     rhs=rk_f[:, r:r + 1],
                                     start=(r == 0), stop=(r == 1))

            idx_u0 = select16(0, w0, per_round=s0_round)
            nc.scalar.dma_start(out=sel_d[:], in_=idx_u0[:])
            outs = [y_d, y2_d]
            for h in range(2):
                offs = small.tile([128, 1], i32, tag=f"offs{h}")
                nc.vector.tensor_scalar(out=offs[:], in0=psum_o[h][:],
                                        scalar1=float(OOB), scalar2=None,
                                        op0=Alu.add)
                nc.gpsimd.indirect_dma_start(
                    out=outs[h][:],
                    out_offset=IndirectOffsetOnAxis(ap=offs[:], axis=0),
                    in_=big[h][:], in_offset=None,
                    bounds_check=BPC * K - 1, oob_is_err=False)

            # ------------- sample 1: reduces + select + gather/store --------
            w1 = emit_reduces(1, dsts1)

            yv1 = y_d[K:2 * K].rearrange("r (u e) -> (r u) e", u=GR)

            def s1_round(r, idx_t8):
                # gather-row index per partition p (rank k = 8r + (p>>4)):
                # (C + idx[k])*16 + (p&15)
                psum_g = psum.tile([128, 1], f32, tag=f"psg{r}")
                nc.tensor.matmul(out=psum_g[:], lhsT=oh8_16[:], rhs=idx_t8[:],
                                 start=True, stop=True)
                grow_f = small.tile([128, 1], f32, tag=f"growf{r}")
                nc.vector.tensor_scalar(out=grow_f[:], in0=psum_g[:],
                                        scalar1=float(GR),
                                        scalar2=float(C * GR), op0=Alu.mult,
                                        op1=Alu.add)
                grow_i = small.tile([128, 1], i32, tag=f"growi{r}")
                nc.vector.tensor_tensor(out=grow_i[:], in0=grow_f[:],
                                        in1=a15f[:], op=Alu.add)
                g = small.tile([128, E // GR], f32, tag=f"g{r}")
                nc.gpsimd.indirect_dma_start(
                    out=g[:], out_offset=None, in_=xg,
                    in_offset=IndirectOffsetOnAxis(ap=grow_i[:], axis=0))
                dma_engines[r].dma_start(out=yv1[128 * r:128 * (r + 1), :],
                                         in_=g[:])

            select16(1, w1, per_round=s1_round)

    nc.compile()
    return nc


def get_nc():
    if "nc" not in _cache:
        _cache["nc"] = _build()
    return _cache["nc"]


def make_in_maps(x: np.ndarray) -> list[dict[str, np.ndarray]]:
    x = np.ascontiguousarray(np.asarray(x, dtype=np.float32))
    assert x.shape == (B, C, H, W)
    return [{"x": x[c * BPC:(c + 1) * BPC].reshape(ROWS, E)}
            for c in range(N_CORES)]


def assemble(results: list[dict[str, np.ndarray]]) -> np.ndarray:
    out = np.empty((B, K, H, W), dtype=np.float32)
    for c in range(N_CORES):
        y = results[c]["y"].reshape(BPC, K, H, W).copy()
        y2 = results[c]["y2"].reshape(BPC, K, H, W)
        sel = results[c]["sel"][0]          # [16] uint32, sample-0 channels
        hi = sel >= 128                      # ranks whose channel is in half 1
        y[0, hi] = y2[0, hi]
        out[c * BPC:(c + 1) * BPC] = y
    return out


def kernel(x: np.ndarray) -> np.ndarray:
    nc = get_nc()
    res = run_bass_kernel_spmd(nc, make_in_maps(x), list(range(N_CORES)))
    return assemble(res.results)



# revision 1
# speedup vs baseline: 1.0114x; 1.0114x over previous
"""Bass/Trainium2 kernel for nn_Channel_attention (bottom-16 channel gather).

reference semantics (per sample b):
    weight = mean(x[b], axis=(H, W))           # [C]
    idx    = argsort(weight)[:16]              # ascending pooled value
    out[b] = x[b, idx]                         # [16, H, W]

Strategy: pure data parallel, B=16 sharded 2 samples per core over 8 cores.
Per core (x shard viewed as [512, 16384] = [(sample, channel), H*W]):

  Sample 0 streams into two RESIDENT [128, 16384] SBUF tiles (16 MiB).
  None of its ops ever wait on a buffer-recycle semaphore, so the Tile
  scheduler keeps its load -> reduce -> select chain first and the DMA
  queues are never head-of-line blocked.  Its output is written mid-run
  with two SWDGE indirect SCATTERS (SBUF -> DRAM, out_offset = output
  rank for selected channels, OOB-skipped otherwise), which avoids the
  1 MiB gather re-read entirely.  The two scatters target two different
  DRAM tensors ("y" and "y2") so no write-write dependency serializes
  them; the host merges rows using the stored selection indices ("sel").
  Scatter offsets are built per max8 round with an accumulating PE
  matmul so round 1's share of the work hides under round 2's DVE ops.

  Sample 1 streams through a rotating pool ([128, 2048] tiles, bufs=7);
  by the time its buffer-recycle semaphores matter, Vector has nothing
  queued but sample-1 reduces, so the recycling never throttles DMA.
  Its selection finishes last, so its 16 channels are re-fetched with
  two full-width [128, 1024] SWDGE indirect gathers (one per max8
  round, 8 ch x 16 sub-rows each; round 1's gather+store overlaps round
  2's select) and stored with direct DMAs.  The last half's chunks
  shrink so the final reduce exits quickly after the last load lands.
"""

import sys

if "/opt/trn_rl_repo" not in sys.path:
    sys.path.insert(0, "/opt/trn_rl_repo")

import numpy as np

from concourse import bacc, mybir, tile
from concourse.bass import IndirectOffsetOnAxis
from concourse.bass_utils import run_bass_kernel_spmd
from concourse.masks import make_identity

N_CORES = 8
B, C, H, W = 16, 256, 128, 128
K = 16
BPC = B // N_CORES          # samples per core = 2
E = H * W                   # 16384 elems per channel
GR = 16                     # gather sub-rows per channel (16 x 4KiB)
ROWS = BPC * C              # 512 channel rows per core
OOB = 1024                  # scatter offset for unselected channels

f32 = mybir.dt.float32
i32 = mybir.dt.int32
u32 = mybir.dt.uint32
X = mybir.AxisListType.X
Alu = mybir.AluOpType

CHUNKS_S0 = [4096] * 4                                # resident, unthrottled
CHUNKS_S1H0 = [2048] * 8                              # pooled
CHUNKS_S1H1 = [2048] * 7 + [1024, 512, 512]           # small tail chunks

_cache = {}


class _FastExitTileContext(tile.TileContext):
    """TileContext whose epilogue skips the per-semaphore clear pass.

    The standard exit drains DMA, barriers, then zeroes every allocated
    semaphore one by one (~250 sems x ~30 ns = ~7 us on the critical
    path).  That clear only matters when another kernel follows in the
    same loaded program; this kernel is single-shot per runtime session,
    so we keep the drain + one all-engine barrier and drop the clears.
    """

    def _drain_and_barrier(self, tick_clock, wait_clock):
        from concourse.vector_clock import ScopedClock

        drain_inst = self.nc.sync.drain()
        wait_clock.add_sem_waits(
            drain_inst.ins, ScopedClock({None: tick_clock.global_clock})
        )
        self.nc.all_engine_barrier()
        popped = self.nc._tile_sem_poison_stack.pop()
        assert popped is self._sem_poison


def _build():
    nc = bacc.Bacc("TRN2", target_bir_lowering=False, debug=False,
                   num_devices=N_CORES)
    x_d = nc.dram_tensor("x", [ROWS, E], f32, kind="ExternalInput")
    y_d = nc.dram_tensor("y", [BPC * K, E], f32, kind="ExternalOutput")
    y2_d = nc.dram_tensor("y2", [BPC * K, E], f32, kind="ExternalOutput")
    sel_d = nc.dram_tensor("sel", [1, K], u32, kind="ExternalOutput")

    with _FastExitTileContext(nc) as tc:
        with (
            tc.tile_pool(name="load", bufs=7) as load_pool,
            tc.tile_pool(name="small", bufs=1) as small,
            tc.tile_pool(name="psum", bufs=1, space="PSUM") as psum,
        ):
            # ---- constants (no deps; scheduler fills gaps with these) ----
            ident = small.tile([128, 128], f32)
            make_identity(nc, ident[:])

            # [8, 128] row iota 0..127 (f32) and (p>>4) variant
            row_i = small.tile([8, 128], i32)
            nc.gpsimd.iota(out=row_i[:], pattern=[[1, 128]], base=0,
                           channel_multiplier=0)
            row_f = small.tile([8, 128], f32)
            nc.vector.tensor_copy(row_f[:], row_i[:])
            rowd16_i = small.tile([8, 128], i32)
            nc.vector.tensor_scalar(out=rowd16_i[:], in0=row_i[:], scalar1=4,
                                    scalar2=None, op0=Alu.arith_shift_right)
            rowd16_f = small.tile([8, 128], f32)
            nc.vector.tensor_copy(rowd16_f[:], rowd16_i[:])

            # [8, 1] partition iota (f32)
            col8_i = small.tile([8, 1], i32)
            nc.gpsimd.iota(out=col8_i[:], pattern=[[1, 1]], base=0,
                           channel_multiplier=1)
            col8_f = small.tile([8, 1], f32)
            nc.vector.tensor_copy(col8_f[:], col8_i[:])

            # oh8_16[j, p] = (p>>4 == j): expands 8 ranks to 128 gather rows
            oh8_16 = small.tile([8, 128], f32)
            nc.vector.tensor_scalar(out=oh8_16[:], in0=rowd16_f[:],
                                    scalar1=col8_f[:], scalar2=None,
                                    op0=Alu.is_equal)

            # [128, 1] (p & 15) as f32, for gather sub-row offsets
            pp = small.tile([128, 1], i32)
            nc.gpsimd.iota(out=pp[:], pattern=[[1, 1]], base=0,
                           channel_multiplier=1)
            nc.vector.tensor_scalar(out=pp[:], in0=pp[:], scalar1=GR - 1,
                                    scalar2=None, op0=Alu.bitwise_and)
            a15f = small.tile([128, 1], f32)
            nc.vector.tensor_copy(a15f[:], pp[:])

            # rank columns for scatter offsets: rk_f[j, r] = 8r + j - OOB
            rk_f = small.tile([8, 2], f32)
            for r in range(2):
                nc.vector.tensor_scalar(out=rk_f[:, r:r + 1], in0=col8_f[:],
                                        scalar1=float(8 * r - OOB),
                                        scalar2=None, op0=Alu.add)

            xg = x_d[:].rearrange("r (u e) -> (r u) e", u=GR)
            dma_engines = [nc.sync, nc.scalar]
            state = {"n_dma": 0}

            # resident tiles for sample 0 (one per 128-channel half)
            big0 = small.tile([128, E], f32, tag="big0")
            big1 = small.tile([128, E], f32, tag="big1")
            big = [big0, big1]

            def chunk_lists(s):
                if s == 0:
                    return [CHUNKS_S0, CHUNKS_S0]
                return [CHUNKS_S1H0, CHUNKS_S1H1]

            def emit_loads(s, into_resident):
                dsts = [[], []]
                for h in range(2):
                    base = s * C + h * 128
                    off = 0
                    for cw in chunk_lists(s)[h]:
                        if into_resident:
                            dst = big[h][:, off:off + cw]
                        else:
                            t = load_pool.tile([128, 2048], f32)
                            dst = t[:, 0:cw]
                        eng = dma_engines[state["n_dma"] % 2]
                        state["n_dma"] += 1
                        eng.dma_start(out=dst,
                                      in_=x_d[base:base + 128, off:off + cw])
                        dsts[h].append(dst)
                        off += cw
                return dsts

            def emit_reduces(s, dsts):
                ncols = max(len(cl) for cl in chunk_lists(s))
                partials = small.tile([128, 2 * ncols], f32,
                                      tag=f"partials{s}")
                sums = small.tile([128, 2], f32, tag=f"sums{s}")
                psum_w = psum.tile([1, C], f32, tag=f"psw{s}")
                w_neg = small.tile([1, C], f32, tag=f"wneg{s}")
                for h in range(2):
                    cl = chunk_lists(s)[h]
                    for j, dst in enumerate(dsts[h]):
                        nc.vector.reduce_sum(
                            out=partials[:, h * ncols + j:h * ncols + j + 1],
                            in_=dst, axis=X)
                    nc.vector.reduce_sum(
                        out=sums[:, h:h + 1],
                        in_=partials[:, h * ncols:h * ncols + len(cl)],
                        axis=X, negate=True)
                    nc.tensor.matmul(out=psum_w[:, h * 128:(h + 1) * 128],
                                     lhsT=sums[:, h:h + 1], rhs=ident[:],
                                     start=True, stop=True)
                    nc.vector.tensor_copy(w_neg[:, h * 128:(h + 1) * 128],
                                          psum_w[:, h * 128:(h + 1) * 128])
                return w_neg

            def select16(s, w_neg, per_round):
                """Two max8 rounds on -sums.  per_round(r, idx_t8) runs after
                each round with that round's transposed indices [8, 1] f32.
                Returns idx_u [1, 16] u32 (ascending pooled sum)."""
                m1 = small.tile([1, 8], f32, tag=f"m1_{s}")
                m2 = small.tile([1, 8], f32, tag=f"m2_{s}")
                w_rep = small.tile([1, C], f32, tag=f"wrep{s}")
                idx_u = small.tile([1, K], u32, tag=f"idxu{s}")

                def round_tail(r):
                    idx_f = small.tile([1, 8], f32, tag=f"idxf{s}_{r}")
                    nc.vector.tensor_copy(idx_f[:],
                                          idx_u[:, 8 * r:8 * r + 8])
                    psum_t = psum.tile([8, 1], f32, tag=f"pst{s}")
                    nc.tensor.matmul(out=psum_t[:], lhsT=idx_f[:],
                                     rhs=ident[0:1, 0:1], start=True,
                                     stop=True)
                    idx_t8 = small.tile([8, 1], f32, tag=f"idxt{s}_{r}")
                    nc.vector.tensor_copy(idx_t8[:], psum_t[:])
                    per_round(r, idx_t8)

                nc.vector.max(out=m1[:], in_=w_neg[:])
                nc.vector.max_index(out=idx_u[:, 0:8], in_max=m1[:],
                                    in_values=w_neg[:])
                nc.vector.match_replace(out=w_rep[:], in_to_replace=m1[:],
                                        in_values=w_neg[:], imm_value=-1e38)
                round_tail(0)
                nc.vector.max(out=m2[:], in_=w_rep[:])
                nc.vector.max_index(out=idx_u[:, 8:16], in_max=m2[:],
                                    in_values=w_rep[:])
                round_tail(1)
                return idx_u

            # ------------- sample 0: resident loads + reduces ---------------
            dsts0 = emit_loads(0, into_resident=True)
            w0 = emit_reduces(0, dsts0)

            # sample 1 pooled load issues queue next on sync/scalar
            dsts1 = emit_loads(1, into_resident=False)

            # ------------- sample 0: select + mid-run scatters --------------
            psum_o0 = psum.tile([128, 1], f32, tag="pso0")
            psum_o1 = psum.tile([128, 1], f32, tag="pso1")
            psum_o = [psum_o0, psum_o1]

            def s0_round(r, idx_t8):
                # accumulate per-half scatter offsets:
                # offs[p] = rank(p) - OOB contribution if channel p selected
                for h in range(2):
                    if h == 0:
                        idx_cmp = idx_t8
                    else:
                        idx_cmp = small.tile([8, 1], f32, tag=f"idxsh{r}")
                        nc.vector.tensor_scalar(out=idx_cmp[:], in0=idx_t8[:],
                                                scalar1=-128.0, scalar2=None,
                                                op0=Alu.add)
                    oh = small.tile([8, 128], f32, tag=f"oh{h}_{r}")
                    nc.vector.tensor_scalar(out=oh[:], in0=row_f[:],
                                            scalar1=idx_cmp[:], scalar2=None,
                                            op0=Alu.is_equal)
                    nc.tensor.matmul(out=psum_o[h][:], lhsT=oh[:],
                                     rhs=rk_f[:, r:r + 1],
                                     start=(r == 0), stop=(r == 1))

            idx_u0 = select16(0, w0, per_round=s0_round)
            nc.scalar.dma_start(out=sel_d[:], in_=idx_u0[:])
            outs = [y_d, y2_d]
            for h in range(2):
                offs = small.tile([128, 1], i32, tag=f"offs{h}")
                nc.vector.tensor_scalar(out=offs[:], in0=psum_o[h][:],
                                        scalar1=float(OOB), scalar2=None,
                                        op0=Alu.add)
                nc.gpsimd.indirect_dma_start(
                    out=outs[h][:],
                    out_offset=IndirectOffsetOnAxis(ap=offs[:], axis=0),
                    in_=big[h][:], in_offset=None,
                    bounds_check=BPC * K - 1, oob_is_err=False)

            # ------------- sample 1: reduces + select + gather/store --------
            w1 = emit_reduces(1, dsts1)

            yv1 = y_d[K:2 * K].rearrange("r (u e) -> (r u) e", u=GR)

            def s1_round(r, idx_t8):
                # gather-row index per partition p (rank k = 8r + (p>>4)):
                # (C + idx[k])*16 + (p&15)
                psum_g = psum.tile([128, 1], f32, tag=f"psg{r}")
                nc.tensor.matmul(out=psum_g[:], lhsT=oh8_16[:], rhs=idx_t8[:],
                                 start=True, stop=True)
                grow_f = small.tile([128, 1], f32, tag=f"growf{r}")
                nc.vector.tensor_scalar(out=grow_f[:], in0=psum_g[:],
                                        scalar1=float(GR),
                                        scalar2=float(C * GR), op0=Alu.mult,
                                        op1=Alu.add)
                grow_i = small.tile([128, 1], i32, tag=f"growi{r}")
                nc.vector.tensor_tensor(out=grow_i[:], in0=grow_f[:],
                                        in1=a15f[:], op=Alu.add)
                g = small.tile([128, E // GR], f32, tag=f"g{r}")
                nc.gpsimd.indirect_dma_start(
                    out=g[:], out_offset=None, in_=xg,
                    in_offset=IndirectOffsetOnAxis(ap=grow_i[:], axis=0))
                dma_engines[r].dma_start(out=yv1[128 * r:128 * (r + 1), :],
                                         in_=g[:])

            select16(1, w1, per_round=s1_round)

    nc.compile()
    return nc


def get_nc():
    if "nc" not in _cache:
        _cache["nc"] = _build()
    return _cache["nc"]


def make_in_maps(x: np.ndarray) -> list[dict[str, np.ndarray]]:
    x = np.ascontiguousarray(np.asarray(x, dtype=np.float32))
    assert x.shape == (B, C, H, W)
    return [{"x": x[c * BPC:(c + 1) * BPC].reshape(ROWS, E)}
            for c in range(N_CORES)]


def assemble(results: list[dict[str, np.ndarray]]) -> np.ndarray:
    out = np.empty((B, K, H, W), dtype=np.float32)
    for c in range(N_CORES):
        y = results[c]["y"].reshape(BPC, K, H, W).copy()
        y2 = results[c]["y2"].reshape(BPC, K, H, W)
        sel = results[c]["sel"][0]          # [16] uint32, sample-0 channels
        hi = sel >= 128                      # ranks whose channel is in half 1
        y[0, hi] = y2[0, hi]
        out[c * BPC:(c + 1) * BPC] = y
    return out


def kernel(x: np.ndarray) -> np.ndarray:
    nc = get_nc()
    res = run_bass_kernel_spmd(nc, make_in_maps(x), list(range(N_CORES)))
    return assemble(res.results)

